# revision 25
# baseline (speedup 1.0000x reference)
"""Trainium2 Bass kernel for NemotronFlash Mamba decoder layer (v4).

Sharding: 8 cores = 2 batches x 4 sequence shards of 512 tokens.
All compute is shard-local except the SSD inter-chunk state, which is
exchanged via one AllGather of (L_k, D_k) within each 4-core batch group.

v4 scheduling changes vs v3:
- rmsnorm1 deferred: raw h is transposed immediately; the per-token
  1/rms scale is folded into every in-projection PSUM drain (rsb row).
- SSD decay-matrix prep (mt) interleaved with the z in-proj groups and
  the collective window; mt kept in SBUF (no DRAM staging).
- Ct never materialized standalone: exp(acs) is broadcast via DMA per
  half-chunk (ebn) and multiplied by C in place.
- z silu issued right after the z drains; gated-norm squares moved to
  gpsimd; the gated-norm scale is folded into the out-proj PSUM drain
  (rsT), letting out-proj pipeline with the Y loop per chunk.
- wo prefetched into the freed wi slots; weight loads spread across the
  sync/scalar/gpsimd DMA queues.
"""
import sys
import numpy as np

sys.path.insert(0, "/opt/trn_rl_repo")

from contextlib import ExitStack  # noqa: E402
import ml_dtypes  # noqa: E402
import concourse.bass as bass  # noqa: E402
import concourse.mybir as mybir  # noqa: E402
import concourse.tile as tile  # noqa: E402
from concourse import bacc  # noqa: E402
from concourse.bass_utils import run_bass_kernel_spmd  # noqa: E402

F32 = mybir.dt.float32
BF16 = mybir.dt.bfloat16
AF = mybir.ActivationFunctionType
OP = mybir.AluOpType

H = 1024
E = 2048
NH = 32
P = 64
NST = 128          # d_state
KC = 4             # d_conv
Q = 128            # chunk len
FF = 4096
CONV = E + 2 * NST          # 2304
D_IN = 2 * E + 2 * NST + NH  # 4384
EPS = 1e-6
NEPS = 1e-5
LSEQ = 512         # tokens per shard
NCHUNK = LSEQ // Q  # 4
NROW = 5           # 5 row tiles of 128 = 640 padded rows
HALO = 3
NCORES = 8

NZT = E // Q       # 16 z tiles
NXT = CONV // Q    # 18 xBC tiles
NMT = 35           # in-proj M tiles (16 z + 18 xBC + 1 dt)
NKH = H // Q       # 8 k tiles over H
NKE = E // Q       # 16 k tiles over E
NFT = FF // Q      # 32 FF tiles


def row_bcast(ap_row, parts=128):
    """AP broadcasting a [1, n] row across `parts` partitions (step-0)."""
    return bass.AP(tensor=ap_row.tensor, offset=ap_row.offset,
                   ap=[[0, parts]] + [list(x) for x in ap_row.ap[1:]])


def colbc(src_ap, n, rep):
    # [128, n, rep] broadcast of per-head columns along a new inner axis
    return bass.AP(tensor=src_ap.tensor, offset=src_ap.offset,
                   ap=[list(src_ap.ap[0])] + [[1, n], [0, rep]])


def rowbc(src_ap, rep, n):
    # [128, rep, n] broadcast of a [128, n] tile along the middle axis
    return bass.AP(tensor=src_ap.tensor, offset=src_ap.offset,
                   ap=[list(src_ap.ap[0])] + [[0, rep], [1, n]])


def build_program(dvals):
    nc = bacc.Bacc("TRN2", target_bir_lowering=False, debug=False,
                   num_devices=NCORES)

    hs_in = nc.dram_tensor("hs", [NROW * 128, H], F32, kind="ExternalInput")
    wiT = nc.dram_tensor("wiT", [9 * 128, NKH * 512], BF16,
                         kind="ExternalInput")
    woT = nc.dram_tensor("woT", [E, H], BF16, kind="ExternalInput")
    wgT = nc.dram_tensor("wgT", [NFT * 128, NKH * 128], BF16,
                         kind="ExternalInput")
    wuT = nc.dram_tensor("wuT", [NFT * 128, NKH * 128], BF16,
                         kind="ExternalInput")
    wdT = nc.dram_tensor("wdT", [FF, H], BF16, kind="ExternalInput")
    wconvd = nc.dram_tensor("wconvd", [128, NXT * KC * 128], BF16,
                            kind="ExternalInput")
    bconv = nc.dram_tensor("bconv", [128, NXT], F32, kind="ExternalInput")
    avec = nc.dram_tensor("avec", [NH, 1], F32, kind="ExternalInput")
    dtb = nc.dram_tensor("dtb", [NH, 1], F32, kind="ExternalInput")
    mask8 = nc.dram_tensor("mask8", [128, 8], F32, kind="ExternalInput")
    negmask = nc.dram_tensor("negmask", [128, 128], F32, kind="ExternalInput")
    idf32 = nc.dram_tensor("idf32", [128, 128], F32, kind="ExternalInput")
    dcol_in = nc.dram_tensor("dcol", [128, NZT], F32, kind="ExternalInput")
    out_d = nc.dram_tensor("out", [LSEQ, H], F32, kind="ExternalOutput")

    with tile.TileContext(nc) as tc, ExitStack() as stack:
        consts = stack.enter_context(tc.tile_pool(name="consts", bufs=1))
        bconv_sb = consts.tile([128, NXT], F32)
        nc.gpsimd.dma_start(out=bconv_sb[:], in_=bconv[:])
        avec_sb = consts.tile([NH, 1], F32)
        nc.gpsimd.dma_start(out=avec_sb[:], in_=avec[:])
        dtb_sb = consts.tile([NH, 1], F32)
        nc.gpsimd.dma_start(out=dtb_sb[:], in_=dtb[:])
        mask_sb = consts.tile([128, 8], F32)
        nc.gpsimd.dma_start(out=mask_sb[:], in_=mask8[:])
        nm_sb = consts.tile([128, 128], F32)
        nc.gpsimd.dma_start(out=nm_sb[:], in_=negmask[:])
        id_sb = consts.tile([128, 128], F32)
        nc.gpsimd.dma_start(out=id_sb[:], in_=idf32[:])
        dcol_sb = consts.tile([128, NZT], F32)
        nc.gpsimd.dma_start(out=dcol_sb[:], in_=dcol_in[:])
        rsb = consts.tile([128, NROW * 128], BF16)
        ones_bf = consts.tile([128, 1], BF16)
        nc.vector.memset(ones_bf[:], 1.0)
        zero32 = consts.tile([NH, Q], F32)
        nc.vector.memset(zero32[:], 0.0)
        epsc = consts.tile([128, 1], F32)
        nc.vector.memset(epsc[:], EPS)
        nepsc = consts.tile([128, 1], F32)
        nc.vector.memset(nepsc[:], NEPS)

        ccdram = stack.enter_context(
            tc.tile_pool(name="ccdram", bufs=1, space="DRAM"))
        cc_in = ccdram.tile([128, E + 1], BF16)
        cc_out = ccdram.tile([4, 128, E + 1], BF16)
        acsR_d = ccdram.tile([NCHUNK * NH, Q], F32)
        acst_d = ccdram.tile([1, NCHUNK * 2 * NH], F32)
        drow_d = ccdram.tile([1, 4 * NH], F32)
        rs1_d = ccdram.tile([1, NROW * 128], F32)
        E2_d = ccdram.tile([NCHUNK * NH, Q], BF16)
        rsrow_d = ccdram.tile([1, LSEQ], F32)

        # Workspace pool; tag-chained slot reuse (bufs=1 per tag):
        #   szs: szT(16K) -> gu_b(16K)
        #   xcs: xc(16K) -> wo_h1(16K)
        #   hts: hT(10K) -> gt(16K)
        #   xbs: xbc(18.5K) -> S_sb(16K) -> h2nT(8K)
        #   csy: cstates(16K) -> gu_a(16K)
        #   xth: x_tm(16K) -> h2(16K f32)
        #   xdw: xdt_all(16K)
        ws = stack.enter_context(tc.tile_pool(name="ws", bufs=1))
        szT = ws.tile([128, NZT, LSEQ], BF16, tag="szs")
        xc = ws.tile([128, NZT, LSEQ], BF16, tag="xcs")
        hT = ws.tile([128, NKH, NROW * 128], BF16, tag="hts")

        es_cf = ExitStack()                        # C .. end of Y loop
        pCF = es_cf.enter_context(tc.tile_pool(name="pCF", bufs=1))
        dtacsT = pCF.tile([128, NCHUNK, 2 * NH], F32)
        alast = pCF.tile([128, NCHUNK, NH], F32)
        wdtb = pCF.tile([128, NCHUNK, NH], BF16)
        dtb16 = pCF.tile([128, NCHUNK, NH], BF16)
        dcstb = pCF.tile([128, NCHUNK, NH], BF16)
        xcbc = pCF.tile([128, 2, LSEQ], BF16)
        G_sb = pCF.tile([128, NCHUNK, Q], BF16)
        B_tm = pCF.tile([128, NCHUNK, NST], BF16)

        es_wip = ExitStack()
        wip = es_wip.enter_context(tc.tile_pool(name="wip", bufs=2))
        mtp = ExitStack()
        mtpool = mtp.enter_context(tc.tile_pool(name="mtpool", bufs=4))
        ebn_es = ExitStack()
        ebnp = ebn_es.enter_context(tc.tile_pool(name="ebnp", bufs=3))
        es_psb = ExitStack()
        psB = es_psb.enter_context(
            tc.tile_pool(name="psB", bufs=2, space="PSUM"))

        es_ct = ExitStack()                        # dt scratch, dies after C
        pCtmp = es_ct.enter_context(tc.tile_pool(name="pCtmp", bufs=1))
        dt_sb = pCtmp.tile([NH, LSEQ], F32)
        acs = pCtmp.tile([NH, LSEQ], F32)
        dtraw = pCtmp.tile([NH, LSEQ], F32)

        # ---------------- Phase A: raw h^T + rms stats ----------
        with tc.tile_pool(name="pA", bufs=1) as pA, \
             tc.tile_pool(name="stat", bufs=1) as stat:
            hsth = {}
            for r in range(NROW):
                for hf in range(2):
                    t = pA.tile([128, 512], F32, tag="hsth", bufs=4,
                                name=f"hst{r}_{hf}")
                    (nc.scalar, nc.gpsimd)[hf].dma_start(
                        out=t[:],
                        in_=hs_in[r * 128:(r + 1) * 128,
                                  hf * 512:(hf + 1) * 512])
                    hbf = pA.tile([128, 512], BF16, tag="hbf", bufs=3)
                    nc.vector.tensor_copy(hbf[:], t[:])
                    nc.sync.dma_start_transpose(
                        hT[:, 4 * hf:4 * hf + 4, r * 128:(r + 1) * 128],
                        hbf[:])
                    sq = pA.tile([128, 512], F32, tag="sq", bufs=2)
                    ssh = stat.tile([128, 1], F32, tag=f"ss{r}_{hf}",
                                    name=f"ss{r}_{hf}")
                    nc.scalar.activation(out=sq[:], in_=t[:],
                                         func=AF.Square,
                                         accum_out=ssh[:])
                    hsth[(r, hf)] = ssh
            rss = []
            for r in range(NROW):
                rs = stat.tile([128, 1], F32, tag=f"rs{r}", name=f"rs{r}")
                nc.vector.tensor_add(rs[:], hsth[(r, 0)][:],
                                     hsth[(r, 1)][:])
                rss.append(rs)
            for r in range(NROW):
                nc.scalar.activation(out=rss[r][:], in_=rss[r][:],
                                     func=AF.Ln, scale=1.0 / H,
                                     bias=epsc[:])
            for r in range(NROW):
                nc.scalar.activation(out=rss[r][:], in_=rss[r][:],
                                     func=AF.Exp, scale=-0.5)
            for r in range(NROW):
                nc.sync.dma_start(out=rs1_d[0:1, r * 128:(r + 1) * 128],
                                  in_=rss[r][:, 0:1])
            nc.gpsimd.dma_start(out=rsb[:], in_=row_bcast(rs1_d[0:1, :]))
            for k in range(NKH):
                nc.vector.tensor_mul(hT[:, k, :], hT[:, k, :], rsb[:])

        cstates = ws.tile([128, NCHUNK, E], BF16, tag="csy")
        x_tm = ws.tile([128, NCHUNK, E], BF16, tag="xth")
        xbc = ws.tile([128, NXT, HALO + LSEQ], BF16, tag="xbs")

        # ---------------- Phase B: in-projection ----------------
        groups = []
        m = 0
        while m < NMT:
            g0 = m
            cols = 0
            while m < NMT and cols + (32 if m == NMT - 1 else 128) <= 512:
                cols += (32 if m == NMT - 1 else 128)
                m += 1
            groups.append((g0, m, cols))

        def do_group(gi, wip, psB, psBh, weng=None):
            g0, g1, cols = groups[gi]
            wi_g = wip.tile([128, NKH, 512], BF16, tag="wi")
            base = g0 * 128
            nc.scalar.dma_start(
                out=wi_g[:, 0:4, :],
                in_=wiT[gi * 128:(gi + 1) * 128, 0:4 * 512])
            nc.gpsimd.dma_start(
                out=wi_g[:, 4:8, :],
                in_=wiT[gi * 128:(gi + 1) * 128, 4 * 512:8 * 512])
            for mm in range(g0, g1):
                mrows = 32 if mm == NMT - 1 else 128
                moff = mm * 128 - base
                ps = psB.tile([128, LSEQ], F32, tag="ps")
                for k in range(NKH):
                    nc.tensor.matmul(
                        ps[:mrows, :],
                        wi_g[:, k, moff:moff + mrows],
                        hT[:, k, HALO:HALO + LSEQ],
                        start=(k == 0), stop=(k == NKH - 1))
                if mm < NZT:
                    nc.vector.tensor_copy(szT[:, mm, :], ps[:])
                elif mm < NZT + NXT:
                    j = mm - NZT
                    nc.vector.tensor_copy(xbc[:, j, HALO:], ps[:])
                    psh = psBh.tile([128, HALO], F32, tag="psh")
                    for k in range(NKH):
                        nc.tensor.matmul(
                            psh[:], wi_g[:, k, moff:moff + 128],
                            hT[:, k, 0:HALO],
                            start=(k == 0), stop=(k == NKH - 1))
                    nc.vector.tensor_copy(xbc[:, j, 0:HALO], psh[:])
                else:
                    nc.vector.tensor_copy(dtraw[:], ps[:NH, :])

        with tc.tile_pool(name="psBh", bufs=1, space="PSUM") as psBh, \
             tc.tile_pool(name="wcd", bufs=3) as wcd, \
             tc.tile_pool(name="psD", bufs=2, space="PSUM") as psD:

            def do_conv(j0, j1):
                for j in range(j0, j1):
                    wc_j = wcd.tile([128, KC, 128], BF16, tag="wc")
                    nc.sync.dma_start(
                        out=wc_j[:],
                        in_=wconvd[:, j * KC * 128:(j + 1) * KC * 128])
                    psc = psD.tile([128, LSEQ], F32, tag="psc")
                    for k in range(KC):
                        nc.tensor.matmul(
                            psc[:], wc_j[:, k, :], xbc[:, j, k:k + LSEQ],
                            start=(k == 0), stop=(k == KC - 1))
                    xdst = (xc[:, j, :] if j < NZT
                            else xcbc[:, j - NZT, :])
                    nc.scalar.activation(out=xdst, in_=psc[:],
                                         func=AF.Silu,
                                         bias=bconv_sb[:, j:j + 1])
                for j in range(j0, min(j1, NZT)):
                    nc.sync.dma_start_transpose(
                        x_tm[:, :, j * 128:(j + 1) * 128], xc[:, j, :])

            do_group(4, wip, psB, psBh)
            do_conv(0, 4)
            for gi in (5, 6, 7):
                do_group(gi, wip, psB, psBh)
                do_conv((gi - 4) * 4, (gi - 4) * 4 + 4)
            do_group(8, wip, psB, psBh)

            # ------------- Phase C: dt path -------------
            with tc.tile_pool(name="pC", bufs=2) as pC, \
                 tc.tile_pool(name="psC", bufs=1, space="PSUM") as psC:
                nc.scalar.activation(out=dtraw[:], in_=dtraw[:],
                                     func=AF.Exp, bias=dtb_sb[:])
                nc.vector.tensor_scalar_add(dtraw[:], dtraw[:], 1.0)
                nc.scalar.activation(out=dt_sb[:], in_=dtraw[:],
                                     func=AF.Ln)
                nc.vector.tensor_scalar_mul(dtraw[:], dt_sb[:],
                                            avec_sb[:])
                for c in range(NCHUNK):
                    nc.vector.tensor_tensor_scan(
                        acs[:, c * Q:(c + 1) * Q],
                        dtraw[:, c * Q:(c + 1) * Q],
                        zero32[:], 0.0, OP.add, OP.add)
                av = acsR_d[:]
                nc.sync.dma_start(
                    out=bass.AP(tensor=av.tensor, offset=av.offset,
                                ap=[[Q, NH], [NH * Q, NCHUNK], [1, Q]]),
                    in_=acs[:].rearrange("h (c q) -> h c q", c=NCHUNK))
                asum = pC.tile([NH, 1], F32, tag="asum")
                nc.vector.tensor_reduce(
                    asum[:],
                    acs[:].rearrange("p (c q) -> p c q",
                                     c=NCHUNK)[:, :, Q - 1],
                    axis=mybir.AxisListType.X, op=OP.add)
                dkcol = pC.tile([NH, 1], BF16, tag="dkcol")
                nc.scalar.activation(out=dkcol[:], in_=asum[:], func=AF.Exp)
                nc.gpsimd.dma_start(out=cc_in[0:NH, E:E + 1], in_=dkcol[:])
                E_sb = pC.tile([NH, LSEQ], BF16, tag="esb")
                nc.scalar.activation(out=E_sb[:], in_=acs[:], func=AF.Exp)
                ev = E2_d[:]
                nc.sync.dma_start(
                    out=bass.AP(tensor=ev.tensor, offset=ev.offset,
                                ap=[[Q, NH], [NH * Q, NCHUNK], [1, Q]]),
                    in_=E_sb[:].rearrange("h (c q) -> h c q", c=NCHUNK))
                for c in range(NCHUNK):
                    pst = psC.tile([128, NH], F32, tag="pst")
                    nc.tensor.transpose(pst[:],
                                        dt_sb[:, c * Q:(c + 1) * Q],
                                        id_sb[0:NH, 0:NH])
                    nc.vector.tensor_copy(dtacsT[:, c, 0:NH], pst[:])
                    pst2 = psC.tile([128, NH], F32, tag="pst2")
                    nc.tensor.transpose(pst2[:],
                                        acs[:, c * Q:(c + 1) * Q],
                                        id_sb[0:NH, 0:NH])
                    nc.vector.tensor_copy(dtacsT[:, c, NH:2 * NH],
                                          pst2[:])
                nc.sync.dma_start(out=acst_d[:],
                                  in_=dtacsT[127:128, :, :])
                at_ = acst_d[:]
                nc.sync.dma_start(
                    out=alast[:],
                    in_=bass.AP(tensor=at_.tensor, offset=at_.offset + NH,
                                ap=[[0, 128], [2 * NH, NCHUNK], [1, NH]]))

            do_conv(16, NXT)
            nc.sync.dma_start_transpose(B_tm[:], xcbc[:, 0, :])
            # decay factors (batched exps, after conv silus)
            with tc.tile_pool(name="pC2", bufs=1) as pC2:
                dec0 = pC2.tile([128, NCHUNK, NH], F32, tag="dec0")
                nc.vector.scalar_tensor_tensor(
                    out=dec0[:], in0=dtacsT[:, :, NH:2 * NH], scalar=-1.0,
                    in1=alast[:], op0=OP.mult, op1=OP.add)
                decT = pC2.tile([128, NCHUNK, NH], F32, tag="decT")
                nc.scalar.activation(out=decT[:], in_=dec0[:], func=AF.Exp)
                nc.scalar.activation(out=dcstb[:], in_=alast[:], func=AF.Exp)
                nc.vector.tensor_mul(wdtb[:], decT[:], dtacsT[:, :, 0:NH])
                nc.vector.tensor_copy(dtb16[:], dtacsT[:, :, 0:NH])
            # G gram matrices (needs xcbc from conv j=16,17)
            with tc.tile_pool(name="psGm", bufs=2, space="PSUM") as psGm:
                for c in range(NCHUNK):
                    gps = psGm.tile([128, Q], F32, tag="gps")
                    nc.tensor.matmul(gps[:], xcbc[:, 0, c * Q:(c + 1) * Q],
                                     xcbc[:, 1, c * Q:(c + 1) * Q],
                                     start=True, stop=True)
                    nc.vector.tensor_mul(G_sb[:, c, :], gps[:], nm_sb[:])

            # ------------ Phase E: states + collective --------
            xv = [x_tm[:, c, :].rearrange("p (h q) -> p h q", h=NH)
                  for c in range(NCHUNK)]
            HG = 8
            NG = NH // HG
            es_pe = ExitStack()
            psE = es_pe.enter_context(
                tc.tile_pool(name="psE", bufs=2, space="PSUM"))
            pE = es_pe.enter_context(tc.tile_pool(name="pE", bufs=3))
            for g in range(NG):
                for c in range(NCHUNK):
                    xdd = pE.tile([128, HG, P], BF16, tag="xdd")
                    nc.vector.tensor_mul(
                        xdd[:],
                        x_tm[:, c, g * 512:(g + 1) * 512].rearrange(
                            "p (h q) -> p h q", h=HG),
                        colbc(wdtb[:, c, g * HG:(g + 1) * HG], HG, P))
                    ps_st = psE.tile([128, 512], F32, tag="ps_st")
                    nc.tensor.matmul(
                        ps_st[:], B_tm[:, c, :], xdd[:],
                        start=True, stop=True)
                    if (g + c) % 2 == 0:
                        nc.scalar.copy(
                            cstates[:, c, g * 512:(g + 1) * 512], ps_st[:])
                    else:
                        nc.vector.tensor_copy(
                            cstates[:, c, g * 512:(g + 1) * 512], ps_st[:])
            # L combine from zero init (bf16, in-place accumulator)
            Lacc = pE.tile([128, E], BF16, tag="lacc", bufs=1)
            nc.vector.tensor_copy(Lacc[:], cstates[:, 0, :])
            for c in range(1, NCHUNK):
                nc.vector.tensor_mul(
                    Lacc[:].rearrange("p (h q) -> p h q", h=NH),
                    Lacc[:].rearrange("p (h q) -> p h q", h=NH),
                    colbc(dcstb[:, c, :], NH, P))
                nc.vector.tensor_add(
                    Lacc[:].rearrange("p (h q) -> p h q", h=NH),
                    Lacc[:].rearrange("p (h q) -> p h q", h=NH),
                    cstates[:, c, :].rearrange("p (h q) -> p h q", h=NH))
            nc.gpsimd.dma_start(out=cc_in[:, 0:E], in_=Lacc[:])
            nc.gpsimd.collective_compute(
                "AllGather", OP.bypass,
                replica_groups=[[0, 1, 2, 3], [4, 5, 6, 7]],
                ins=[cc_in.opt()], outs=[cc_out.opt()])
            es_pe.close()

        es_ct.close()

        # ---------------- xdt build (gpsimd, pre-collective) -----------
        xdt_all = ws.tile([128, NCHUNK, E], BF16, tag="xdw")
        for c in range(NCHUNK):
            nc.gpsimd.tensor_mul(
                xdt_all[:, c, :].rearrange("p (h q) -> p h q", h=NH),
                xv[c], colbc(dtb16[:, c, :], NH, P))

        # ------- gap phase: z groups interleaved with mt prep ----------
        mts = []
        ct_done = set()
        ebns = {}

        def load_ebn(c, half):
            ebn = ebnp.tile([128, 16, Q], BF16, tag="ebn",
                            name=f"ebn{c}_{half}")
            ed = E2_d[:]
            nc.sync.dma_start(
                out=ebn[:],
                in_=bass.AP(tensor=ed.tensor,
                            offset=ed.offset + (c * NH + half * 16) * Q,
                            ap=[[0, 128], [1, 16 * Q]]))
            ebns[(c, half)] = ebn

        S_sb = ws.tile([128, NCHUNK, E], BF16, tag="xbs")
        with tc.tile_pool(name="psSeg", bufs=3, space="PSUM") as psSeg:

            abp = ExitStack()
            acsbcp = abp.enter_context(tc.tile_pool(name="acsbcp",
                                                    bufs=1))
            abengs = (nc.sync, nc.scalar, nc.gpsimd)

            def prep_mt(c):
                mt_c = mtpool.tile([128, NH, Q], BF16, tag="mtc",
                                   name=f"mt{c}")
                mts.append(mt_c)
                for half in range(2):
                    hb = half * 16
                    ab = acsbcp.tile([128, 16, Q], F32, tag="ab")
                    av2 = acsR_d[:]
                    abengs[(2 * c + half) % 3].dma_start(
                        out=ab[:],
                        in_=bass.AP(tensor=av2.tensor,
                                    offset=av2.offset + (c * NH + hb) * Q,
                                    ap=[[0, 128], [1, 16 * Q]]))
                    for qi in range(2):
                        h0 = hb + qi * 8
                        sps = psSeg.tile([128, 8, Q], F32, tag="sps")
                        nc.vector.tensor_sub(
                            sps[:], ab[:, qi * 8:(qi + 1) * 8, :],
                            colbc(dtacsT[:, c, NH + h0:NH + h0 + 8],
                                  8, Q))
                        nc.vector.tensor_scalar_min(sps[:], sps[:], 0.0)
                        nc.scalar.activation(
                            out=mt_c[:, h0:h0 + 8, :], in_=sps[:],
                            func=AF.Exp)
                geng = nc.vector if c < 2 else nc.gpsimd
                geng.tensor_mul(
                    mt_c[:], mt_c[:], rowbc(G_sb[:, c, :], NH, Q))

            for c in range(2):
                do_group(c, wip, psB, None)
                prep_mt(c)
            do_group(2, wip, psB, None)
            do_group(3, wip, psB, None)
            # z silu: batched, one table load
            for cc in range(NCHUNK):
                nc.scalar.activation(
                    out=szT[:, :, cc * Q:(cc + 1) * Q],
                    in_=szT[:, :, cc * Q:(cc + 1) * Q],
                    func=AF.Silu)

            # early ebn loads + Ct muls (run during the collective wait)
            for (cc, hh) in ((0, 0), (0, 1), (1, 0)):
                load_ebn(cc, hh)
                ceng = nc.vector if hh == 0 else nc.gpsimd
                ceng.tensor_mul(
                    ebns[(cc, hh)][:], ebns[(cc, hh)][:],
                    rowbc(xcbc[:, 1, cc * Q:(cc + 1) * Q], 16, Q))
                ct_done.add((cc, hh))

            # ---------- S_init combine + S recurrence (during cc wait) --
            with tc.tile_pool(name="pS", bufs=1) as pS:
                Dg = pS.tile([NH, 4], BF16, tag="Dg")
                for jj in range(4):
                    nc.sync.dma_start(out=Dg[:, jj:jj + 1],
                                      in_=cc_out[jj, 0:NH, E:E + 1])
                deff = pS.tile([NH, 4], F32, tag="deff")
                for jj in range(4):
                    nc.vector.scalar_tensor_tensor(
                        out=deff[:, jj:jj + 1], in0=Dg[:, jj:jj + 1],
                        scalar=mask_sb[0:NH, jj:jj + 1],
                        in1=mask_sb[0:NH, 4 + jj:5 + jj],
                        op0=OP.mult, op1=OP.add)
                for jj in range(4):
                    nc.sync.dma_start(
                        out=drow_d[0:1, jj * NH:(jj + 1) * NH],
                        in_=deff[:, jj:jj + 1])
                dbc = pS.tile([128, 4 * NH], F32, tag="dbc")
                nc.sync.dma_start(out=dbc[:],
                                  in_=row_bcast(drow_d[0:1, :]))
                Sacc = pS.tile([128, E], BF16, tag="sacc", bufs=1)
                for jj in range(4):
                    Lgj = pS.tile([128, E], BF16, tag="Lg", bufs=2,
                                  name=f"Lg{jj}")
                    nc.sync.dma_start(out=Lgj[:], in_=cc_out[jj, :, 0:E])
                    if jj == 0:
                        nc.vector.tensor_scalar_mul(Sacc[:], Lgj[:],
                                                    mask_sb[:, 0:1])
                    else:
                        nc.vector.tensor_mul(
                            Sacc[:].rearrange("p (h q) -> p h q", h=NH),
                            Sacc[:].rearrange("p (h q) -> p h q", h=NH),
                            colbc(dbc[:, jj * NH:(jj + 1) * NH], NH, P))
                        nc.vector.scalar_tensor_tensor(
                            out=Sacc[:], in0=Lgj[:],
                            scalar=mask_sb[:, jj:jj + 1], in1=Sacc[:],
                            op0=OP.mult, op1=OP.add)
                nc.vector.tensor_copy(S_sb[:, 0, :], Sacc[:])
                for c in range(NCHUNK - 1):
                    nc.vector.tensor_mul(
                        S_sb[:, c + 1, :].rearrange("p (h q) -> p h q",
                                                    h=NH),
                        S_sb[:, c, :].rearrange("p (h q) -> p h q", h=NH),
                        colbc(dcstb[:, c, :], NH, P))
                    nc.vector.tensor_add(
                        S_sb[:, c + 1, :], S_sb[:, c + 1, :],
                        cstates[:, c, :])

            prep_mt(2)
            prep_mt(3)
            abp.close()
        es_psb.close()

        # Y-phase pools (opened before pS so closes stay LIFO)
        es_y = ExitStack()
        psY = es_y.enter_context(
            tc.tile_pool(name="psY", bufs=4, space="PSUM"))
        psN = es_y.enter_context(
            tc.tile_pool(name="psN", bufs=1, space="PSUM"))
        psO = es_y.enter_context(
            tc.tile_pool(name="psO", bufs=2, space="PSUM"))
        pYd = es_y.enter_context(tc.tile_pool(name="pYd", bufs=3))
        pGa = es_y.enter_context(tc.tile_pool(name="pGa", bufs=2))
        stat2 = es_y.enter_context(tc.tile_pool(name="stat2", bufs=4))
        sqps = psN.tile([128, LSEQ], F32)
        rsT = pGa.tile([128, NCHUNK], F32, tag="rsT", bufs=1)

        # wo half0 prefetch into the freed wi slots (two 8k tiles)
        wt = woT[:]
        wo_h0a = wip.tile([128, 8, 512], BF16, tag="wi", name="wo_h0a")
        nc.sync.dma_start(
            out=wo_h0a[:],
            in_=bass.AP(tensor=wt.tensor, offset=wt.offset,
                        ap=[[H, 128], [128 * H, 8], [1, 512]]))
        wo_h0b = wip.tile([128, 8, 512], BF16, tag="wi", name="wo_h0b")
        nc.gpsimd.dma_start(
            out=wo_h0b[:],
            in_=bass.AP(tensor=wt.tensor, offset=wt.offset + 1024 * H,
                        ap=[[H, 128], [128 * H, 8], [1, 512]]))

        def wo_h0(k):
            return wo_h0a[:, k, :] if k < 8 else wo_h0b[:, k - 8, :]

        pend = [(1, 1), (2, 0), (2, 1), (3, 0), (3, 1)]

        # ------- Y: chunk-major mms + drains + pipelined out-proj -------
        gt = ws.tile([128, NKE, LSEQ], BF16, tag="hts")
        h2 = ws.tile([128, NCHUNK, H], F32, tag="xth")

        def do_y(c):
            mt_c = mts[c]
            for half in range(2):
                ebn_c = ebns[(c, half)]
                if (c, half) not in ct_done:
                    ceng = nc.vector if half == 0 else nc.gpsimd
                    ceng.tensor_mul(
                        ebn_c[:], ebn_c[:],
                        rowbc(xcbc[:, 1, c * Q:(c + 1) * Q], 16, Q))
                for j in range(half * 8, half * 8 + 8):
                    psy = psY.tile([128, Q], F32, tag="psy")
                    for hh in range(2):
                        h = 2 * j + hh
                        out_ap = psy[hh * 64:(hh + 1) * 64, :]
                        nc.tensor.matmul(
                            out_ap,
                            xdt_all[:, c, h * P:(h + 1) * P],
                            mt_c[:, h, :],
                            start=True, stop=False)
                        nc.tensor.matmul(
                            out_ap,
                            S_sb[:, c, h * P:(h + 1) * P],
                            ebn_c[:, h - 16 * half, :],
                            start=False, stop=True)
                    ydr = pYd.tile([128, Q], BF16, tag="ydr")
                    nc.vector.scalar_tensor_tensor(
                        out=ydr[:], in0=xc[:, j, c * Q:(c + 1) * Q],
                        scalar=dcol_sb[:, j:j + 1], in1=psy[:],
                        op0=OP.mult, op1=OP.add)
                    nc.vector.tensor_mul(gt[:, j, c * Q:(c + 1) * Q],
                                         ydr[:],
                                         szT[:, j, c * Q:(c + 1) * Q])
                    g2 = pYd.tile([128, Q], BF16, tag="g2")
                    nc.gpsimd.tensor_mul(g2[:],
                                         gt[:, j, c * Q:(c + 1) * Q],
                                         gt[:, j, c * Q:(c + 1) * Q])
                    nc.tensor.matmul(sqps[0:1, c * Q:(c + 1) * Q],
                                     ones_bf[:], g2[:],
                                     start=(j == 0), stop=(j == NZT - 1))
                if pend:
                    load_ebn(*pend.pop(0))

        def do_rs(c0, c1):
            # rsT[:, c] = (mean_E(gt^2) + eps)^-1/2 per token
            n = (c1 - c0) * Q
            rsr = pYd.tile([1, n], F32, tag="rsr", bufs=2)
            nc.scalar.activation(out=rsr[:], in_=sqps[0:1, c0 * Q:c1 * Q],
                                 func=AF.Ln, scale=1.0 / E,
                                 bias=nepsc[0:1, :])
            nc.scalar.activation(out=rsr[:], in_=rsr[:],
                                 func=AF.Exp, scale=-0.5)
            nc.sync.dma_start(out=rsrow_d[0:1, c0 * Q:c1 * Q], in_=rsr[:])
            rd = rsrow_d[:]
            nc.sync.dma_start(
                out=rsT[:, c0:c1],
                in_=bass.AP(tensor=rd.tensor, offset=rd.offset + c0 * Q,
                            ap=[[1, 128], [128, c1 - c0]]))

        def do_op(tt, half, wo_src):
            ps = psO.tile([128, 512], F32, tag="po")
            for k in range(NKE):
                nc.tensor.matmul(
                    ps[:], gt[:, k, tt * 128:(tt + 1) * 128],
                    wo_src(k),
                    start=(k == 0), stop=(k == NKE - 1))
            hsr = pGa.tile([128, 512], F32, tag="hsr")
            nc.sync.dma_start(
                out=hsr[:],
                in_=hs_in[HALO + tt * 128:HALO + (tt + 1) * 128,
                          half * 512:(half + 1) * 512])
            nc.vector.scalar_tensor_tensor(
                out=h2[:, tt, half * 512:(half + 1) * 512],
                in0=ps[:], scalar=rsT[:, tt:tt + 1], in1=hsr[:],
                op0=OP.mult, op1=OP.add)

        h2nT = ws.tile([128, NKH, LSEQ], BF16, tag="xbs")

        def rms2_round(tts):
            ss2s, rs2s = [], []
            for tt in tts:
                sq2 = pGa.tile([128, H], F32, tag="sq2", bufs=1)
                ss2 = stat2.tile([128, 1], F32, tag=f"ss2{tt}",
                                 name=f"ss2{tt}")
                nc.scalar.activation(out=sq2[:], in_=h2[:, tt, :],
                                     func=AF.Square, accum_out=ss2[:])
                ss2s.append(ss2)
            for i, tt in enumerate(tts):
                rs2 = stat2.tile([128, 1], F32, tag=f"rs2{tt}",
                                 name=f"rs2{tt}")
                nc.scalar.activation(out=rs2[:], in_=ss2s[i][:],
                                     func=AF.Ln, scale=1.0 / H,
                                     bias=epsc[:])
                rs2s.append(rs2)
            for i in range(2):
                nc.scalar.activation(out=rs2s[i][:], in_=rs2s[i][:],
                                     func=AF.Exp, scale=-0.5)
            for i, tt in enumerate(tts):
                h2n = pGa.tile([128, H], BF16, tag="h2n")
                nc.vector.tensor_scalar_mul(h2n[:], h2[:, tt, :],
                                            rs2s[i][:])
                teng = (nc.sync, nc.scalar)[i]
                teng.dma_start_transpose(
                    h2nT[:, :, tt * 128:(tt + 1) * 128], h2n[:])

        do_y(0)
        do_y(1)
        do_rs(0, 2)
        do_op(0, 0, wo_h0)
        do_y(2)
        do_op(1, 0, wo_h0)
        do_y(3)
        # wo half1 into the xc slot (xc dead after Y)
        wo_h1 = ws.tile([128, NKE, 512], BF16, tag="xcs", name="wo_h1")
        for k in range(NKE):
            weng = (nc.sync, nc.gpsimd, nc.scalar)[k % 3]
            weng.dma_start(
                out=wo_h1[:, k, :],
                in_=woT[k * 128:(k + 1) * 128, 512:1024])
        do_rs(2, 4)
        do_op(2, 0, wo_h0)
        do_op(3, 0, wo_h0)
        do_op(0, 1, lambda k: wo_h1[:, k, :])
        do_op(1, 1, lambda k: wo_h1[:, k, :])
        rms2_round((0, 1))
        do_op(2, 1, lambda k: wo_h1[:, k, :])
        do_op(3, 1, lambda k: wo_h1[:, k, :])
        rms2_round((2, 3))
        es_y.close()
        ebn_es.close()
        mtp.close()
        es_wip.close()
        es_cf.close()

        # ---------------- Phase H: MLP ----------------
        gu_a = ws.tile([128, NFT // 2, LSEQ], BF16, tag="csy")
        gu_b = ws.tile([128, NFT // 2, LSEQ], BF16, tag="szs")

        def gu_t(mf):
            return (gu_a[:, mf, :] if mf < NFT // 2
                    else gu_b[:, mf - NFT // 2, :])

        with tc.tile_pool(name="wmP", bufs=6) as wmP, \
             tc.tile_pool(name="psM", bufs=4, space="PSUM") as psM, \
             tc.tile_pool(name="pM", bufs=3) as pM:
            for mf in range(NFT):
                wg_m = wmP.tile([128, NKH, 128], BF16, tag="wg")
                nc.sync.dma_start(out=wg_m[:],
                                  in_=wgT[mf * 128:(mf + 1) * 128, :])
                wu_m = wmP.tile([128, NKH, 128], BF16, tag="wu")
                nc.gpsimd.dma_start(out=wu_m[:],
                                    in_=wuT[mf * 128:(mf + 1) * 128, :])
                gps = psM.tile([128, LSEQ], F32, tag="gps")
                for k in range(NKH):
                    nc.tensor.matmul(gps[:], wg_m[:, k, :], h2nT[:, k, :],
                                     start=(k == 0), stop=(k == NKH - 1))
                sg = pM.tile([128, LSEQ], BF16, tag="sg")
                nc.scalar.activation(out=sg[:], in_=gps[:], func=AF.Silu)
                ups = psM.tile([128, LSEQ], F32, tag="ups")
                for k in range(NKH):
                    nc.tensor.matmul(ups[:], wu_m[:, k, :], h2nT[:, k, :],
                                     start=(k == 0), stop=(k == NKH - 1))
                nc.vector.tensor_mul(gu_t(mf), sg[:], ups[:])
        with tc.tile_pool(name="wdP", bufs=3) as wdP, \
             tc.tile_pool(name="psD2", bufs=1, space="PSUM") as psD2, \
             tc.tile_pool(name="pO", bufs=4) as pO:
            dps = []
            for i in range(8):
                dpt = psD2.tile([128, 512], F32, tag=f"dp{i}", name=f"dp{i}")
                dps.append(dpt)
            for k in range(NFT):
                wd_k = wdP.tile([128, H], BF16, tag="wd")
                weng = nc.sync if k % 2 == 0 else nc.gpsimd
                weng.dma_start(out=wd_k[:],
                               in_=wdT[k * 128:(k + 1) * 128, :])
                for tt in range(NCHUNK):
                    for half in range(2):
                        nc.tensor.matmul(
                            dps[tt * 2 + half][:],
                            gu_t(k)[:, tt * 128:(tt + 1) * 128],
                            wd_k[:, half * 512:(half + 1) * 512],
                            start=(k == 0), stop=(k == NFT - 1))
            for tt in range(NCHUNK):
                for half in range(2):
                    ob = pO.tile([128, 512], F32, tag="ob")
                    nc.vector.tensor_add(
                        ob[:], dps[tt * 2 + half][:],
                        h2[:, tt, half * 512:(half + 1) * 512])
                    weng = (nc.sync, nc.gpsimd)[half]
                    weng.dma_start(
                        out=out_d[tt * 128:(tt + 1) * 128,
                                  half * 512:(half + 1) * 512],
                        in_=ob[:])

    nc.finalize()
    return nc


_CACHE = {}


def _get_program():
    if "p" not in _CACHE:
        _CACHE["p"] = build_program(None)
    return _CACHE["p"]


def kernel(hidden_states, w_ln1, w_in, w_conv, b_conv, dt_bias, A_log, D,
           w_mnorm, w_out, w_ln2, w_gate, w_up, w_down):
    bf = ml_dtypes.bfloat16
    hs = np.asarray(hidden_states, np.float32)
    wiTn = (np.asarray(w_in, np.float32) *
            np.asarray(w_ln1, np.float32)[None, :]).T.astype(bf)
    # pre-tile [H, D_IN] -> groups of 512 cols: [9*128, NKH*512]
    wi_pad = np.zeros((H, 9 * 512), bf)
    wi_pad[:, 0:D_IN] = wiTn
    wiTn = wi_pad.reshape(NKH, 128, 9, 512).transpose(2, 1, 0, 3) \
        .reshape(9 * 128, NKH * 512)
    woTn = (np.asarray(w_out, np.float32) *
            np.asarray(w_mnorm, np.float32)[None, :]).T.astype(bf)
    wgTn = (np.asarray(w_gate, np.float32) *
            np.asarray(w_ln2, np.float32)[None, :]).T.astype(bf)
    wuTn = (np.asarray(w_up, np.float32) *
            np.asarray(w_ln2, np.float32)[None, :]).T.astype(bf)
    wgTn = wgTn.reshape(NKH, 128, NFT, 128).transpose(2, 1, 0, 3) \
        .reshape(NFT * 128, NKH * 128)
    wuTn = wuTn.reshape(NKH, 128, NFT, 128).transpose(2, 1, 0, 3) \
        .reshape(NFT * 128, NKH * 128)
    wdTn = np.asarray(w_down, np.float32).T.astype(bf)
    # conv as diagonal stationaries: [128, NXT, KC, 128]
    wcr = np.asarray(w_conv, np.float32).reshape(NXT, 128, KC) \
        .transpose(1, 0, 2)                       # [p, j, k]
    wcd = np.zeros((128, NXT, KC, 128), np.float32)
    idx = np.arange(128)
    wcd[idx[:, None, None], np.arange(NXT)[None, :, None],
        np.arange(KC)[None, None, :], idx[:, None, None]] = wcr
    wcd = wcd.astype(bf).reshape(128, NXT * KC * 128)
    bconv = np.asarray(b_conv, np.float32).reshape(NXT, 128).T.copy()
    avec = (-np.exp(np.asarray(A_log, np.float32))).reshape(NH, 1)
    dtb = np.asarray(dt_bias, np.float32).reshape(NH, 1)
    negmask = (np.arange(128)[None, :] >= np.arange(128)[:, None]) \
        .astype(np.float32)
    Dv = np.asarray(D, np.float32)
    dcol = np.zeros((128, NZT), np.float32)
    for j in range(NZT):
        dcol[0:64, j] = Dv[2 * j]
        dcol[64:128, j] = Dv[2 * j + 1]
    idf = np.eye(128, dtype=np.float32)

    nc = _get_program()

    shared = dict(wiT=np.ascontiguousarray(wiTn),
                  woT=np.ascontiguousarray(woTn),
                  wgT=np.ascontiguousarray(wgTn),
                  wuT=np.ascontiguousarray(wuTn),
                  wdT=np.ascontiguousarray(wdTn),
                  wconvd=np.ascontiguousarray(wcd),
                  bconv=bconv, avec=avec, dtb=dtb,
                  negmask=negmask, idf32=idf, dcol=dcol)
    in_maps = []
    for core in range(NCORES):
        b, r = core // 4, core % 4
        s0 = r * LSEQ
        hpad = np.zeros((NROW * 128, H), np.float32)
        hpad[HALO:HALO + LSEQ] = hs[b, s0:s0 + LSEQ]
        if s0 > 0:
            hpad[0:HALO] = hs[b, s0 - HALO:s0]
        m8 = np.zeros((128, 8), np.float32)
        for j in range(4):
            m8[:, j] = 1.0 if j < r else 0.0
            m8[:, 4 + j] = 0.0 if j < r else 1.0
        in_maps.append(dict(shared, hs=hpad, mask8=m8))

    res = run_bass_kernel_spmd(nc, in_maps, list(range(NCORES)))
    out = np.empty((2, 2048, H), np.float32)
    for core in range(NCORES):
        b, r = core // 4, core % 4
        out[b, r * LSEQ:(r + 1) * LSEQ] = res.results[core]["out"]
    return out
